# revision 4
# baseline (speedup 1.0000x reference)
"""Trainium2 Bass kernel for the 2-qubit quantum-circuit batch evaluation.

Reference semantics (per batch row, x = [x0, x1], scalar theta):
    state = RY(theta) @ CNOT @ (RY(x0)|0> (x) RY(x1)|0>)
    out = (<Z> + 1)/2 for each qubit.

Algebraically (product/half-angle identities):
    out0 = 0.5 + 0.5*cos(th)*cos(x0) - 0.5*sin(th)*sin(x0)*sin(x1)
    out1 = 0.5 + 0.5*cos(x0)*cos(x1)

Per element we need sin(x) and cos(x) with |x| up to ~18 while the ACT Sin
is only accurate to |arg| <~ 3.3, so reduce once:
    y = x - 2*pi*round(x/(2*pi))   (magic-constant rounding, fp32)
    sin(x) = Sin(y);  cos(x) = Sin(pi/2 - |y|)

Engine assignment (from measured per-op timings; GPSIMD software fp16/bf16
*writes* are ~16x slow, so Pool only ever writes fp32):
  Pool : t = x/(2pi)+MAGIC ; k = (t-MAGIC)*(-2pi)  (both fp32 out)
         a = 0.5*cos(th)*c0 + 0.5 ; m2 = -0.5*sin(th)*m01 ; out1 =
         0.5*g01 + 0.5  (fp16 in -> fp32 out tensor_scalars, ~full rate)
  DVE  : y = x + k (fp16 out) ; half of |y| via int32-view bitwise AND;
         [m01|g01] = [s0|c0]*[s1|c1] one fp16 2x tensor_tensor;
         out0 = m2 + a (fp32 in, fp16 out)
  ACT  : Sin(y) -> S ; Sin(pi/2 - |y|) -> C ; other half of |y| (Abs)
DMA: fp16 x in (half traffic), fp16 out0 + fp32 out1 out.

Sharding: pure data parallel over 8 cores; host deinterleaves x into packed
x0/x1 free-dim halves (fp16) and reassembles outputs (layout/dtype only).
theta-derived scalars ride a tiny [128, 4] fp32 constant tensor.
"""

import numpy as np

import concourse.bass as bass
import concourse.mybir as mybir
from concourse.alu_op_type import AluOpType
from concourse.bacc import Bacc
from concourse.tile import TileContext
from concourse import bass_utils

N_CORES = 8
B = 8388608
BC = B // N_CORES            # rows per core
P = 128                      # SBUF partitions
F = 4096                     # free elems per partition per tile (x0-half | x1-half)
H = F // 2
T = (BC * 2) // (P * F)      # tiles per core
MAGIC = float(1.5 * 2**23)
INV2PI = float(1.0 / (2.0 * np.pi))
NEG2PI = float(-2.0 * np.pi)
HALFPI = float(np.pi / 2.0)
ABSMASK = 0x7FFF7FFF         # clears fp16 sign bits, two lanes per int32

_CACHE = {}


def _build_nc():
    nc = Bacc()
    f32 = mybir.dt.float32
    f16 = mybir.dt.float16
    i32 = mybir.dt.int32
    x = nc.dram_tensor("x", [T, P, F], f16, kind="ExternalInput")
    consts = nc.dram_tensor("consts", [P, 4], f32, kind="ExternalInput")
    out0 = nc.dram_tensor("out0", [T, P, H], f16, kind="ExternalOutput")
    out1 = nc.dram_tensor("out1", [T, P, H], f32, kind="ExternalOutput")

    Sin = mybir.ActivationFunctionType.Sin
    Abs = mybir.ActivationFunctionType.Abs

    with TileContext(nc) as tc:
        with tc.tile_pool(name="cpool", bufs=1) as cpool, \
             tc.tile_pool(name="io", bufs=2) as io, \
             tc.tile_pool(name="red", bufs=2) as red, \
             tc.tile_pool(name="mid", bufs=2) as mid, \
             tc.tile_pool(name="tail", bufs=2) as tail:
            ct = cpool.tile([P, 4], f32)
            nc.sync.dma_start(out=ct[:], in_=consts[:])
            hc = ct[:, 0:1]      # 0.5*cos(theta)
            ns = ct[:, 1:2]      # -0.5*sin(theta)
            halfpi = ct[:, 2:3]  # pi/2

            for i in range(T):
                xt = io.tile([P, F], f16, tag="xt")
                nc.sync.dma_start(out=xt[:], in_=x[i])

                # range reduction: y = x - 2pi*round(x/2pi)  (y fp16, |y|<=pi)
                # k = (t-MAGIC)*(-2pi) computed in place over t (elementwise).
                t32 = red.tile([P, F], f32, tag="t32")
                y16 = red.tile([P, F], f16, tag="y16")
                nc.gpsimd.tensor_scalar(
                    t32[:], xt[:], INV2PI, MAGIC, AluOpType.mult, AluOpType.add,
                )
                nc.gpsimd.tensor_scalar(
                    t32[:], t32[:], MAGIC, NEG2PI, AluOpType.subtract, AluOpType.mult,
                )
                nc.vector.tensor_tensor(y16[:], xt[:], t32[:], AluOpType.add)

                # |y|: first half on ACT (Abs), second half on DVE (int AND)
                ay = red.tile([P, F], f16, tag="ay")
                nc.scalar.activation(ay[:, 0:H], y16[:, 0:H], Abs)
                nc.vector.tensor_scalar(
                    ay[:, H:F].bitcast(i32), y16[:, H:F].bitcast(i32),
                    ABSMASK, None, AluOpType.bitwise_and,
                )

                # S = sin(y) -> SC[:, 0:F];  C = sin(pi/2 - |y|) -> SC[:, F:2F]
                sc = mid.tile([P, 2 * F], f16, tag="sc")
                nc.scalar.activation(sc[:, 0:F], y16[:], Sin)
                nc.scalar.activation(sc[:, F:], ay[:], Sin, bias=halfpi, scale=-1.0)

                # [m01|g01] = [s0|c0] * [s1|c1]  (one fp16 2x tensor_tensor)
                scv = sc[:].rearrange("p (two f) -> p two f", two=2)
                mg = mid.tile([P, F], f16, tag="mg")
                mgv = mg[:].rearrange("p (two h) -> p two h", two=2)
                nc.vector.tensor_tensor(
                    mgv[:, :, :], scv[:, :, 0:H], scv[:, :, H:F], AluOpType.mult
                )
                m01 = mg[:, 0:H]
                g01 = mg[:, H:F]
                c0 = sc[:, F:F + H]

                # theta affines on Pool (fp32 out), final add on DVE
                a32 = tail.tile([P, H], f32, tag="a32")
                m232 = tail.tile([P, H], f32, tag="m232")
                o0t = tail.tile([P, H], f16, tag="o0t")
                o1t = tail.tile([P, H], f32, tag="o1t")
                nc.gpsimd.tensor_scalar(
                    a32[:], c0, hc, 0.5, AluOpType.mult, AluOpType.add,
                )
                nc.gpsimd.tensor_scalar(
                    m232[:], m01, ns, None, AluOpType.mult,
                )
                nc.vector.tensor_tensor(o0t[:], m232[:], a32[:], AluOpType.add)
                nc.gpsimd.tensor_scalar(
                    o1t[:], g01, 0.5, 0.5, AluOpType.mult, AluOpType.add,
                )
                nc.sync.dma_start(out=out0[i], in_=o0t[:])
                nc.sync.dma_start(out=out1[i], in_=o1t[:])
    nc.compile()
    return nc


def _run(in_maps, trace=False, trace_cores=None):
    if "nc" not in _CACHE:
        _CACHE["nc"] = _build_nc()
    return bass_utils.run_bass_kernel_spmd(
        _CACHE["nc"],
        in_maps,
        core_ids=list(range(N_CORES)),
        trace=trace,
        trace_cores=trace_cores,
    )


def kernel(x, theta, _trace=False, _trace_cores=None):
    x = np.asarray(x)
    theta = np.asarray(theta, dtype=np.float32)
    assert x.shape == (B, 2), x.shape

    th = float(theta.reshape(-1)[0])
    consts = np.empty((P, 4), dtype=np.float32)
    consts[:, 0] = 0.5 * np.cos(th)
    consts[:, 1] = -0.5 * np.sin(th)
    consts[:, 2] = HALFPI
    consts[:, 3] = 0.0

    x16 = x.astype(np.float16)                       # [B, 2]
    x0 = x16[:, 0].reshape(N_CORES, T, P, H)
    x1 = x16[:, 1].reshape(N_CORES, T, P, H)
    xs = np.ascontiguousarray(np.concatenate([x0, x1], axis=3))  # [N,T,P,F]

    in_maps = [{"x": xs[c], "consts": consts} for c in range(N_CORES)]
    res = _run(in_maps, trace=_trace, trace_cores=_trace_cores)
    _CACHE["last_results"] = res

    o0 = np.stack([res.results[c]["out0"] for c in range(N_CORES)])  # [N,T,P,H] f16
    o1 = np.stack([res.results[c]["out1"] for c in range(N_CORES)])  # [N,T,P,H] f32
    out = np.empty((B, 2), dtype=np.float32)
    out[:, 0] = o0.reshape(B).astype(np.float32)
    out[:, 1] = o1.reshape(B)
    return out


# revision 9
# speedup vs baseline: 1.6592x; 1.6592x over previous
"""Trainium2 Bass kernel for the 2-qubit quantum-circuit batch evaluation.

Reference semantics (per batch row, x = [x0, x1], scalar theta):
    state = RY(theta) @ CNOT @ (RY(x0)|0> (x) RY(x1)|0>)
    out = (<Z> + 1)/2 for each qubit.

Algebraically (product/half-angle identities):
    out0 = 0.5 + 0.5*cos(th)*cos(x0) - 0.5*sin(th)*sin(x0)*sin(x1)
    out1 = 0.5 + 0.5*cos(x0)*cos(x1)

Per element we need sin(x) and cos(x) with |x| up to ~18 while the ACT Sin
is only accurate to |arg| <~ 3.3, so reduce once:
    y = x - 2*pi*round(x/(2*pi))   (magic-constant rounding, fp32)
    sin(x) = Sin(y);  cos(x) = Sin(pi/2 - |y|)

Engine assignment (from measured per-op timings; GPSIMD software fp16/bf16
*writes* are ~16x slow, so Pool only ever writes fp32):
  Pool : t = x/(2pi)+MAGIC ; k = (t-MAGIC)*(-2pi)  (both fp32 out)
         a = 0.5*cos(th)*c0 + 0.5 ; m2 = -0.5*sin(th)*m01 ; out1 =
         0.5*g01 + 0.5  (fp16 in -> fp32 out tensor_scalars, ~full rate)
  DVE  : y = x + k (fp16 out) ; half of |y| via int32-view bitwise AND;
         [m01|g01] = [s0|c0]*[s1|c1] one fp16 2x tensor_tensor;
         out0 = m2 + a (fp32 in, fp16 out)
  ACT  : Sin(y) -> S ; Sin(pi/2 - |y|) -> C ; other half of |y| (Abs)
DMA: fp16 x in (half traffic), fp16 out0 + fp32 out1 out.

Sharding: pure data parallel over 8 cores; host deinterleaves x into packed
x0/x1 free-dim halves (fp16) and reassembles outputs (layout/dtype only).
theta-derived scalars ride a tiny [128, 4] fp32 constant tensor.
"""

import numpy as np

import concourse.bass as bass
import concourse.mybir as mybir
from concourse.alu_op_type import AluOpType
from concourse.bacc import Bacc
from concourse.tile import TileContext
from concourse import bass_utils

N_CORES = 8
B = 8388608
BC = B // N_CORES            # rows per core
P = 128                      # SBUF partitions
F = 4096                     # free elems per partition per tile (x0-half | x1-half)
H = F // 2
T = (BC * 2) // (P * F)      # tiles per core
MAGIC = float(1.5 * 2**23)
INV2PI = float(1.0 / (2.0 * np.pi))
NEG2PI = float(-2.0 * np.pi)
HALFPI = float(np.pi / 2.0)
ABSMASK = 0x7FFF7FFF         # clears fp16 sign bits, two lanes per int32
ABS_SPLIT = 3072             # |y| split point: [0:split] ACT Abs, [split:F] DVE AND

_CACHE = {}


def _build_nc():
    nc = Bacc()
    f32 = mybir.dt.float32
    f16 = mybir.dt.float16
    i32 = mybir.dt.int32
    x = nc.dram_tensor("x", [T, P, F], f16, kind="ExternalInput")
    consts = nc.dram_tensor("consts", [P, 4], f32, kind="ExternalInput")
    out0 = nc.dram_tensor("out0", [T, P, H], f16, kind="ExternalOutput")
    out1 = nc.dram_tensor("out1", [T, P, H], f16, kind="ExternalOutput")

    Sin = mybir.ActivationFunctionType.Sin
    Abs = mybir.ActivationFunctionType.Abs

    with TileContext(nc) as tc:
        with tc.tile_pool(name="cpool", bufs=1) as cpool, \
             tc.tile_pool(name="io", bufs=2) as io, \
             tc.tile_pool(name="red", bufs=2) as red, \
             tc.tile_pool(name="mid", bufs=2) as mid, \
             tc.tile_pool(name="tail", bufs=2) as tail:
            ct = cpool.tile([P, 4], f32)
            nc.sync.dma_start(out=ct[:], in_=consts[:])
            hc = ct[:, 0:1]      # 0.5*cos(theta)
            ns = ct[:, 1:2]      # -0.5*sin(theta)
            halfpi = ct[:, 2:3]  # pi/2

            for i in range(T):
                xt = io.tile([P, F], f16, tag="xt")
                nc.sync.dma_start(out=xt[:], in_=x[i])

                # range reduction: y = x - 2pi*round(x/2pi)  (y fp16, |y|<=pi)
                # Pool must write a fresh fp32 tile each op (in-place or
                # 16-bit-out GPSIMD ops fall off a software cliff).
                t32 = red.tile([P, F], f32, tag="t32")
                k32 = red.tile([P, F], f32, tag="k32")
                y16 = red.tile([P, F], f16, tag="y16")
                nc.gpsimd.tensor_scalar(
                    t32[:], xt[:], INV2PI, MAGIC, AluOpType.mult, AluOpType.add,
                )
                nc.gpsimd.tensor_scalar(
                    k32[:], t32[:], MAGIC, NEG2PI, AluOpType.subtract, AluOpType.mult,
                )
                nc.vector.tensor_tensor(y16[:], xt[:], k32[:], AluOpType.add)

                # |y|: 3/4 on ACT (Abs), 1/4 on DVE (int32-view AND), to balance
                ay = red.tile([P, F], f16, tag="ay")
                nc.scalar.activation(ay[:, 0:ABS_SPLIT], y16[:, 0:ABS_SPLIT], Abs)
                nc.vector.tensor_scalar(
                    ay[:, ABS_SPLIT:F].bitcast(i32), y16[:, ABS_SPLIT:F].bitcast(i32),
                    ABSMASK, None, AluOpType.bitwise_and,
                )

                # S = sin(y) -> SC[:, 0:F];  C = sin(pi/2 - |y|) -> SC[:, F:2F]
                sc = mid.tile([P, 2 * F], f16, tag="sc")
                nc.scalar.activation(sc[:, 0:F], y16[:], Sin)
                nc.scalar.activation(sc[:, F:], ay[:], Sin, bias=halfpi, scale=-1.0)

                # [m01|g01] = [s0|c0] * [s1|c1]  (one fp16 2x tensor_tensor)
                scv = sc[:].rearrange("p (two f) -> p two f", two=2)
                mg = mid.tile([P, F], f16, tag="mg")
                mgv = mg[:].rearrange("p (two h) -> p two h", two=2)
                nc.vector.tensor_tensor(
                    mgv[:, :, :], scv[:, :, 0:H], scv[:, :, H:F], AluOpType.mult
                )
                m01 = mg[:, 0:H]
                g01 = mg[:, H:F]
                c0 = sc[:, F:F + H]

                # theta affines: DVE fp16 tensor_scalars run at 4x
                a16 = tail.tile([P, H], f16, tag="a16")
                m216 = tail.tile([P, H], f16, tag="m216")
                o0t = tail.tile([P, H], f16, tag="o0t")
                o1t = tail.tile([P, H], f16, tag="o1t")
                nc.vector.tensor_scalar(
                    a16[:], c0, hc, 0.5, AluOpType.mult, AluOpType.add,
                )
                nc.vector.tensor_scalar(
                    m216[:], m01, ns, 0.0, AluOpType.mult, AluOpType.add,
                )
                nc.vector.tensor_tensor(o0t[:], m216[:], a16[:], AluOpType.add)
                nc.vector.tensor_scalar(
                    o1t[:], g01, 0.5, 0.5, AluOpType.mult, AluOpType.add,
                )
                nc.sync.dma_start(out=out0[i], in_=o0t[:])
                nc.sync.dma_start(out=out1[i], in_=o1t[:])
    nc.compile()
    return nc


def _run(in_maps, trace=False, trace_cores=None):
    if "nc" not in _CACHE:
        _CACHE["nc"] = _build_nc()
    return bass_utils.run_bass_kernel_spmd(
        _CACHE["nc"],
        in_maps,
        core_ids=list(range(N_CORES)),
        trace=trace,
        trace_cores=trace_cores,
    )


def kernel(x, theta, _trace=False, _trace_cores=None):
    x = np.asarray(x)
    theta = np.asarray(theta, dtype=np.float32)
    assert x.shape == (B, 2), x.shape

    th = float(theta.reshape(-1)[0])
    consts = np.empty((P, 4), dtype=np.float32)
    consts[:, 0] = 0.5 * np.cos(th)
    consts[:, 1] = -0.5 * np.sin(th)
    consts[:, 2] = HALFPI
    consts[:, 3] = 0.0

    x16 = x.astype(np.float16)                       # [B, 2]
    x0 = x16[:, 0].reshape(N_CORES, T, P, H)
    x1 = x16[:, 1].reshape(N_CORES, T, P, H)
    xs = np.ascontiguousarray(np.concatenate([x0, x1], axis=3))  # [N,T,P,F]

    in_maps = [{"x": xs[c], "consts": consts} for c in range(N_CORES)]
    res = _run(in_maps, trace=_trace, trace_cores=_trace_cores)
    _CACHE["last_results"] = res

    o0 = np.stack([res.results[c]["out0"] for c in range(N_CORES)])  # [N,T,P,H] f16
    o1 = np.stack([res.results[c]["out1"] for c in range(N_CORES)])  # [N,T,P,H] f16
    out = np.empty((B, 2), dtype=np.float32)
    out[:, 0] = o0.reshape(B).astype(np.float32)
    out[:, 1] = o1.reshape(B).astype(np.float32)
    return out


# revision 10
# speedup vs baseline: 5.5584x; 3.3500x over previous
"""Trainium2 Bass kernel for the 2-qubit quantum-circuit batch evaluation.

Reference semantics (per batch row, x = [x0, x1], scalar theta):
    state = RY(theta) @ CNOT @ (RY(x0)|0> (x) RY(x1)|0>)
    out = (<Z> + 1)/2 for each qubit.

Algebraically (product/half-angle identities):
    out0 = 0.5 + 0.5*cos(th)*cos(x0) - 0.5*sin(th)*sin(x0)*sin(x1)
    out1 = 0.5 + 0.5*cos(x0)*cos(x1)

Per element we need sin(x) and cos(x) with |x| up to ~18 while the ACT Sin
is only accurate to |arg| <~ 3.3, so reduce once:
    y = x - 2*pi*round(x/(2*pi))   (magic-constant rounding, fp32)
    sin(x) = Sin(y);  cos(x) = Sin(pi/2 - |y|)

Engine assignment (from measured per-op timings; GPSIMD software fp16/bf16
*writes* are ~16x slow, so Pool only ever writes fp32):
  Pool : t = x/(2pi)+MAGIC ; k = (t-MAGIC)*(-2pi)  (both fp32 out)
         a = 0.5*cos(th)*c0 + 0.5 ; m2 = -0.5*sin(th)*m01 ; out1 =
         0.5*g01 + 0.5  (fp16 in -> fp32 out tensor_scalars, ~full rate)
  DVE  : y = x + k (fp16 out) ; half of |y| via int32-view bitwise AND;
         [m01|g01] = [s0|c0]*[s1|c1] one fp16 2x tensor_tensor;
         out0 = m2 + a (fp32 in, fp16 out)
  ACT  : Sin(y) -> S ; Sin(pi/2 - |y|) -> C ; other half of |y| (Abs)
DMA: fp16 x in (half traffic), fp16 out0 + fp32 out1 out.

Sharding: pure data parallel over 8 cores; host deinterleaves x into packed
x0/x1 free-dim halves (fp16) and reassembles outputs (layout/dtype only).
theta-derived scalars ride a tiny [128, 4] fp32 constant tensor.
"""

import numpy as np

import concourse.bass as bass
import concourse.mybir as mybir
from concourse.alu_op_type import AluOpType
from concourse.bacc import Bacc
from concourse.tile import TileContext
from concourse import bass_utils

N_CORES = 8
B = 8388608
BC = B // N_CORES            # rows per core
P = 128                      # SBUF partitions
F = 4096                     # free elems per partition per tile (x0-half | x1-half)
H = F // 2
T = (BC * 2) // (P * F)      # tiles per core
MAGIC = float(1.5 * 2**23)
INV2PI = float(1.0 / (2.0 * np.pi))
NEG2PI = float(-2.0 * np.pi)
HALFPI = float(np.pi / 2.0)
ABSMASK = 0x7FFF7FFF         # clears fp16 sign bits, two lanes per int32
ABS_SPLIT = 3072             # |y| split point: [0:split] ACT Abs, [split:F] DVE AND

_CACHE = {}


def _build_nc():
    nc = Bacc()
    f32 = mybir.dt.float32
    f16 = mybir.dt.float16
    i32 = mybir.dt.int32
    x = nc.dram_tensor("x", [T, P, F], f16, kind="ExternalInput")
    consts = nc.dram_tensor("consts", [P, 4], f32, kind="ExternalInput")
    out0 = nc.dram_tensor("out0", [T, P, H], f16, kind="ExternalOutput")
    out1 = nc.dram_tensor("out1", [T, P, H], f16, kind="ExternalOutput")

    Sin = mybir.ActivationFunctionType.Sin
    Abs = mybir.ActivationFunctionType.Abs

    with TileContext(nc) as tc:
        with tc.tile_pool(name="cpool", bufs=1) as cpool, \
             tc.tile_pool(name="io", bufs=2) as io, \
             tc.tile_pool(name="red", bufs=2) as red, \
             tc.tile_pool(name="mid", bufs=2) as mid, \
             tc.tile_pool(name="tail", bufs=2) as tail:
            ct = cpool.tile([P, 4], f32)
            nc.sync.dma_start(out=ct[:], in_=consts[:])
            hc = ct[:, 0:1]      # 0.5*cos(theta)
            ns = ct[:, 1:2]      # -0.5*sin(theta)
            halfpi = ct[:, 2:3]  # pi/2

            for i in range(T):
                xt = io.tile([P, F], f16, tag="xt")
                nc.sync.dma_start(out=xt[:], in_=x[i])

                # range reduction: y = x - 2pi*round(x/2pi)  (y fp16, |y|<=pi)
                # GPSIMD only has a fast path for (MULTIPLY, ADD) fp32-out
                # tensor_scalars, so both Pool ops take that shape and the
                # -2pi scale rides the DVE scalar_tensor_tensor that forms y.
                t32 = red.tile([P, F], f32, tag="t32")
                m32 = red.tile([P, F], f32, tag="m32")
                y16 = red.tile([P, F], f16, tag="y16")
                nc.gpsimd.tensor_scalar(
                    t32[:], xt[:], INV2PI, MAGIC, AluOpType.mult, AluOpType.add,
                )
                nc.gpsimd.tensor_scalar(
                    m32[:], t32[:], 1.0, -MAGIC, AluOpType.mult, AluOpType.add,
                )
                nc.vector.scalar_tensor_tensor(
                    y16[:], m32[:], NEG2PI, xt[:], AluOpType.mult, AluOpType.add,
                )

                # |y|: 3/4 on ACT (Abs), 1/4 on DVE (int32-view AND), to balance
                ay = red.tile([P, F], f16, tag="ay")
                nc.scalar.activation(ay[:, 0:ABS_SPLIT], y16[:, 0:ABS_SPLIT], Abs)
                nc.vector.tensor_scalar(
                    ay[:, ABS_SPLIT:F].bitcast(i32), y16[:, ABS_SPLIT:F].bitcast(i32),
                    ABSMASK, None, AluOpType.bitwise_and,
                )

                # S = sin(y) -> SC[:, 0:F];  C = sin(pi/2 - |y|) -> SC[:, F:2F]
                sc = mid.tile([P, 2 * F], f16, tag="sc")
                nc.scalar.activation(sc[:, 0:F], y16[:], Sin)
                nc.scalar.activation(sc[:, F:], ay[:], Sin, bias=halfpi, scale=-1.0)

                # [m01|g01] = [s0|c0] * [s1|c1]  (one fp16 2x tensor_tensor)
                scv = sc[:].rearrange("p (two f) -> p two f", two=2)
                mg = mid.tile([P, F], f16, tag="mg")
                mgv = mg[:].rearrange("p (two h) -> p two h", two=2)
                nc.vector.tensor_tensor(
                    mgv[:, :, :], scv[:, :, 0:H], scv[:, :, H:F], AluOpType.mult
                )
                m01 = mg[:, 0:H]
                g01 = mg[:, H:F]
                c0 = sc[:, F:F + H]

                # theta affines: DVE fp16 tensor_scalars run at 4x
                a16 = tail.tile([P, H], f16, tag="a16")
                m216 = tail.tile([P, H], f16, tag="m216")
                o0t = tail.tile([P, H], f16, tag="o0t")
                o1t = tail.tile([P, H], f16, tag="o1t")
                nc.vector.tensor_scalar(
                    a16[:], c0, hc, 0.5, AluOpType.mult, AluOpType.add,
                )
                nc.vector.tensor_scalar(
                    m216[:], m01, ns, 0.0, AluOpType.mult, AluOpType.add,
                )
                nc.vector.tensor_tensor(o0t[:], m216[:], a16[:], AluOpType.add)
                nc.vector.tensor_scalar(
                    o1t[:], g01, 0.5, 0.5, AluOpType.mult, AluOpType.add,
                )
                nc.sync.dma_start(out=out0[i], in_=o0t[:])
                nc.sync.dma_start(out=out1[i], in_=o1t[:])
    nc.compile()
    return nc


def _run(in_maps, trace=False, trace_cores=None):
    if "nc" not in _CACHE:
        _CACHE["nc"] = _build_nc()
    return bass_utils.run_bass_kernel_spmd(
        _CACHE["nc"],
        in_maps,
        core_ids=list(range(N_CORES)),
        trace=trace,
        trace_cores=trace_cores,
    )


def kernel(x, theta, _trace=False, _trace_cores=None):
    x = np.asarray(x)
    theta = np.asarray(theta, dtype=np.float32)
    assert x.shape == (B, 2), x.shape

    th = float(theta.reshape(-1)[0])
    consts = np.empty((P, 4), dtype=np.float32)
    consts[:, 0] = 0.5 * np.cos(th)
    consts[:, 1] = -0.5 * np.sin(th)
    consts[:, 2] = HALFPI
    consts[:, 3] = 0.0

    x16 = x.astype(np.float16)                       # [B, 2]
    x0 = x16[:, 0].reshape(N_CORES, T, P, H)
    x1 = x16[:, 1].reshape(N_CORES, T, P, H)
    xs = np.ascontiguousarray(np.concatenate([x0, x1], axis=3))  # [N,T,P,F]

    in_maps = [{"x": xs[c], "consts": consts} for c in range(N_CORES)]
    res = _run(in_maps, trace=_trace, trace_cores=_trace_cores)
    _CACHE["last_results"] = res

    o0 = np.stack([res.results[c]["out0"] for c in range(N_CORES)])  # [N,T,P,H] f16
    o1 = np.stack([res.results[c]["out1"] for c in range(N_CORES)])  # [N,T,P,H] f16
    out = np.empty((B, 2), dtype=np.float32)
    out[:, 0] = o0.reshape(B).astype(np.float32)
    out[:, 1] = o1.reshape(B).astype(np.float32)
    return out
